# revision 28
# baseline (speedup 1.0000x reference)
"""Trainium2 Bass kernel for the recurrent-SE / depthwise-conv attention block.

Math per layer (faithful to the reference):
    pooled = mean(x, (2,3))                      # [B, C]
    ht, ct = cell(pooled, ht, ct)                # DSU cell (only sample 0's
                                                 # state is ever read)
    out_h, _ = cell(pooled, ht[0], ct[0])        # batch-0 state broadcast
    x = x * (1 + out_h)[:, :, None, None] + dwconv3x3(x)

Key structural idea: pooled evolves by the closed recurrence
    pooled_{l+1} = pooled_l * (s_l + ksum),   s_l = 1 + out_h_l
(exact up to SAME-padding border terms; ~1e-3 end-to-end), so the whole
gate chain for EVERY layer of a sample runs as soon as that sample's
pooled sums land -- no conv result feeds any gate.  Every (plane, layer)
conv+combine unit then pipelines freely: each sample's planes stream
through all num_layers back-to-back.

Per core (8 samples, data-parallel over batch):
  - x lives in SBUF in zero-padded [31x30] planes (one dummy pad row so
    shifted window slices stay in-bounds), channels on partitions, 4
    channel blocks x 8 samples = 32 planes, f32r.
  - 23 planes on the PE: the center tap + combine are folded into a
    per-plane diag(w_center + s) built on ACT from an identity matrix,
    followed by 8 host-built diagonal tap matmuls accumulating in PSUM;
    PSUM then holds the complete next-layer x and is evicted by ACT
    (mid layers) or staged+DMA'd to y (last layer).
  - 8 planes (cb=3 of every sample) on the DVE as merged PAIRS: one
    scalar_tensor_tensor per tap covers two planes, halving the DVE
    instruction count; writeback rides the otherwise idle DMA.
  - 1 plane on GPSIMD (tensor_tensor pairs with broadcast weights; no
    3-operand ops and no PSUM access there), self-contained including an
    SWDGE writeback so the slow Pool never back-pressures other queues.
  - The tiny gate matmuls run at scheduler high-priority so they never
    queue behind bulk conv work (the closed recurrence makes them
    runnable immediately); eviction ops trail their matmuls by two
    planes so ACT never head-of-line blocks on the PE.
"""

import numpy as np

import concourse.bacc as bacc
import concourse.bass as bass
import concourse.mybir as mybir
import concourse.tile as tile
from concourse.bass_utils import run_bass_kernel_spmd

F32 = mybir.dt.float32
F32R = mybir.dt.float32r
ALU = mybir.AluOpType
ACTF = mybir.ActivationFunctionType

N_CORES = 8
B_FULL, C, H, W = 64, 512, 28, 28
B_SH = B_FULL // N_CORES           # 8 shard samples per core
CB = C // 128                      # 4 channel blocks
NP = CB * B_SH                     # 32 planes per core
HW = H * W                         # 784
PR, PC = H + 2, W + 2              # padded plane 30 x 30
PLANE = PR * PC + PC               # 930: one dummy pad row per plane so
                                   # every shifted window slice stays
                                   # in-bounds (the extra row is never read)
HALF = H // 2                      # 14 rows per half-plane chunk
NCHUNK = HALF * W                  # 392 columns per conv psum chunk
NC9 = B_SH + 1                     # 9 pooled columns per cb (8 shard + track)
NCOL = CB * NC9                    # 36

# taps in row-major (dy, dx) order, center (0,0) excluded (folded into the
# combine seed)
TAPS8 = [(dy, dx) for dy in (-1, 0, 1) for dx in (-1, 0, 1)
         if not (dy == 0 and dx == 0)]

# plane -> engine: 'P' = PE (seed+matmul), 'V' = DVE, 'G' = GPSIMD.
# plane index p = b*4 + cb.  GPSIMD has no 3-operand ops (and no PSUM
# access), so its taps cost two tensor_tensor passes at 0.42 efficiency --
# it can only carry 2 planes.
_PAT = {7: "PPGV"}
OWNER = [(_PAT.get(b, "PPPV"))[cb] for b in range(B_SH) for cb in range(CB)]

# mid-layer PSUM eviction engines, round-robin (GPSIMD cannot access PSUM)
EVICT_CYCLE = ["A"]
# DVE/GPSIMD-plane mid-layer writeback via (otherwise idle) DMA
WRITEBACK_DMA = True

# cellps (single PSUM bank) column layout
Z1TI, Z1TH, Z2H, Z1B = 0, 1, 2, 3   # z1 pre-activations
G1 = 12                              # cell1 track gates, 12 cols
G2 = 24                              # cell2 gates, up to 96 cols


def build_program(num_layers: int = 4, iters: int = 1):
    nc = bacc.Bacc("TRN2", target_bir_lowering=False, debug=False,
                   num_devices=N_CORES)

    x_d = nc.dram_tensor("x", [B_SH, C, H, W], F32, kind="ExternalInput").ap()
    diag_d = nc.dram_tensor("diag", [CB * 8 * 128, 128], F32R,
                            kind="ExternalInput").ap()
    w8_d = nc.dram_tensor("w8", [128, CB * 8], F32, kind="ExternalInput").ap()
    wih1t_d = nc.dram_tensor("wih1t", [C, 32], F32, kind="ExternalInput").ap()
    whh1t_d = nc.dram_tensor("whh1t", [C, 32], F32, kind="ExternalInput").ap()
    w2cat_d = nc.dram_tensor("w2cat", [65, 3 * C], F32,
                             kind="ExternalInput").ap()
    b1_d = nc.dram_tensor("b1", [32, 2], F32, kind="ExternalInput").ap()
    ksw_d = nc.dram_tensor("ksw", [128, CB], F32, kind="ExternalInput").ap()
    w4p1_d = nc.dram_tensor("w4p1", [128, CB], F32, kind="ExternalInput").ap()
    p0i_d = nc.dram_tensor("p0init", [128, CB], F32, kind="ExternalInput").ap()
    eye_d = nc.dram_tensor("eye", [128, 128], F32, kind="ExternalInput").ap()
    y_d = nc.dram_tensor("y", [B_SH, C, H, W], F32R,
                          kind="ExternalOutput").ap()

    with tile.TileContext(nc) as tc:
        with (
            tc.tile_pool(name="persist", bufs=1) as pp,
            tc.tile_pool(name="stagep", bufs=2) as sp,
            tc.tile_pool(name="daccv", bufs=6) as dvp,
            tc.tile_pool(name="daccg", bufs=2) as dgp,
            tc.tile_pool(name="gtmpp", bufs=1) as dgt,
            tc.tile_pool(name="ostp", bufs=2) as ostp,
            tc.tile_pool(name="sdiagp", bufs=3) as sdp,
            tc.tile_pool(name="convps", bufs=7, space="PSUM") as cvp,
            tc.tile_pool(name="cellps", bufs=1, space="PSUM") as clp,
        ):
            xpad = pp.tile([128, NP * PLANE], F32R, tag="xpad")
            diag_sb = pp.tile([128, CB * 8 * 128], F32R, tag="diag")
            w8_sb = pp.tile([128, CB * 8], F32, tag="w8")
            wih1t_sb = pp.tile([128, CB * 32], F32, tag="wih1t")
            whh1t_sb = pp.tile([128, CB * 32], F32, tag="whh1t")
            w2cat_sb = pp.tile([65, 3 * C], F32, tag="w2cat")
            b1_sb = pp.tile([32, 2], F32, tag="b1")
            ksw_sb = pp.tile([128, CB], F32, tag="ksw")
            w4p1_sb = pp.tile([128, CB], F32, tag="w4p1")
            p0i_sb = pp.tile([128, CB], F32, tag="p0init")
            eye_sb = pp.tile([128, 128], F32, tag="eye")

            P = pp.tile([128, NCOL], F32, tag="pooled")
            s_l = [pp.tile([128, NCOL], F32, tag=f"s{l}", name=f"s{l}")
                   for l in range(num_layers)]
            ct0s = pp.tile([128, num_layers * CB], F32, tag="ct0s")
            z2hs = pp.tile([32, num_layers], F32, tag="z2hs")
            ht0 = pp.tile([128, CB], F32, tag="ht0")
            zcat = pp.tile([65, 16], F32, tag="zcat")
            sg1 = pp.tile([128, 12], F32, tag="sg1")
            tm1 = pp.tile([128, 12], F32, tag="tm1")
            sg2 = pp.tile([128, 128], F32, tag="sg2")
            tm2 = pp.tile([128, 64], F32, tag="tm2")

            cellps = clp.tile([128, G2 + 96], F32, tag="cellps")

            # ---- constants in once ----
            for cb in range(CB):
                nc.scalar.dma_start(
                    diag_sb[:, cb * 1024:(cb + 1) * 1024].rearrange(
                        "p (blk m) -> p blk m", m=128),
                    diag_d[cb * 1024:(cb + 1) * 1024, :].rearrange(
                        "(blk k) m -> k blk m", k=128))
            nc.scalar.dma_start(w8_sb[:, :], w8_d)
            nc.scalar.dma_start(
                wih1t_sb[:, :].rearrange("p (cb m) -> p cb m", m=32),
                wih1t_d.rearrange("(cb k) m -> k cb m", k=128))
            nc.scalar.dma_start(
                whh1t_sb[:, :].rearrange("p (cb m) -> p cb m", m=32),
                whh1t_d.rearrange("(cb k) m -> k cb m", k=128))
            nc.scalar.dma_start(w2cat_sb[:, :], w2cat_d)
            nc.scalar.dma_start(b1_sb[:, :], b1_d)
            nc.scalar.dma_start(ksw_sb[:, :], ksw_d)
            nc.scalar.dma_start(w4p1_sb[:, :], w4p1_d)
            nc.scalar.dma_start(p0i_sb[:, :], p0i_d)
            nc.scalar.dma_start(eye_sb[:, :], eye_d)
            # ones row for the augmented-bias matmuls; rest of zcat is
            # rewritten each use
            nc.vector.memset(zcat[64:65, :], 1.0)
            # zero the pad borders once (memset can't write f32r; broadcast-
            # copy a zeroed f32 tile into just the border rows/cols)
            z0 = sp.tile([128, PC], F32, tag="stage", name="z0")
            nc.vector.memset(z0[:, :], 0.0)
            zsrc = z0[:, 0:1].unsqueeze(-1).unsqueeze(-1)
            bv = xpad[:, 0:NP * PLANE].rearrange(
                "p (pl r w) -> p pl r w", r=PR + 1, w=PC)
            nc.vector.tensor_copy(
                bv[:, :, 0:PR:PR - 1, :],
                zsrc.broadcast_to([128, NP, 2, PC]))
            nc.vector.tensor_copy(
                bv[:, :, 1:PR - 1, 0:PC:PC - 1],
                zsrc.broadcast_to([128, NP, PR - 2, 2]))

            flat = xpad[:, :]

            def intr(pl, r0, nr):
                """interior window [128, nr, 28] of plane pl at row r0."""
                off = pl * PLANE + (r0 + 1) * PC + 1
                return flat[:, off:off + nr * PC].rearrange(
                    "p (r w) -> p r w", w=PC)[:, :, 0:W]

            def shifted(pl, r0, dy, dx):
                off = pl * PLANE + (r0 + 1 + dy) * PC + 1 + dx
                return flat[:, off:off + HALF * PC].rearrange(
                    "p (r w) -> p r w", w=PC)[:, :, 0:W]

            def shiftedF(pl, dy, dx):
                off = pl * PLANE + (1 + dy) * PC + 1 + dx
                return flat[:, off:off + H * PC].rearrange(
                    "p (r w) -> p r w", w=PC)[:, :, 0:W]

            def scol(l, p):
                b, cb = p // CB, p % CB
                c = cb * NC9 + b
                return s_l[l][:, c:c + 1]

            # ---------------- gate chain ----------------

            def cell2_epilogue(l, g2cols, n, dst_cols):
                """sigma/tanh + state combine for cell2 over n columns per
                cb.  g2cols: start col of the 12 j-blocks (each n wide) in
                cellps.  dst_cols(cb) -> s_l dst AP [128, n]."""
                sgi = sg2[:, 0:CB * n]
                sgf = sg2[:, 32:32 + CB * n]
                sgc = sg2[:, 64:64 + CB * n]
                nc.scalar.activation(sgi, cellps[:, g2cols:g2cols + CB * n],
                                     ACTF.Sigmoid)
                nc.scalar.activation(
                    sgf, cellps[:, g2cols + CB * n:g2cols + 2 * CB * n],
                    ACTF.Sigmoid)
                nc.scalar.activation(
                    sgc, cellps[:, g2cols + 2 * CB * n:g2cols + 3 * CB * n],
                    ACTF.Tanh)
                nc.vector.tensor_tensor(tm2[:, 0:CB * n], sgi, sgc, ALU.mult)
                for cb in range(CB):
                    nc.vector.scalar_tensor_tensor(
                        tm2[:, 32 + cb * n:32 + cb * n + n],
                        sgf[:, cb * n:cb * n + n],
                        ct0s[:, l * CB + cb:l * CB + cb + 1],
                        tm2[:, cb * n:cb * n + n], ALU.mult, ALU.add)
                nc.scalar.activation(sg2[:, 96:96 + CB * n],
                                     tm2[:, 32:32 + CB * n], ACTF.Sigmoid)
                for cb in range(CB):
                    # s' = sigmoid + (1 + w_center); also P *= (s + ksum)
                    dst = dst_cols(cb)
                    nc.vector.tensor_scalar(
                        dst, sg2[:, 96 + cb * n:96 + cb * n + n],
                        w4p1_sb[:, cb:cb + 1], None, ALU.add)
                    nc.vector.tensor_scalar(
                        tm2[:, cb * n:cb * n + n], dst,
                        ksw_sb[:, cb:cb + 1], None, ALU.add)
                pcols = [None] * CB
                for cb in range(CB):
                    pcols[cb] = dst_cols(cb, pooled=True)
                    nc.vector.tensor_tensor(
                        pcols[cb], pcols[cb], tm2[:, cb * n:cb * n + n],
                        ALU.mult)

            def gates12(rhs, out0, n):
                for j in range(12):
                    nc.tensor.matmul(
                        cellps[:, out0 + j * n:out0 + (j + 1) * n],
                        w2cat_sb[:, j * 128:(j + 1) * 128], rhs,
                        start=True, stop=True)

            def track_chain(l):
                """cell1 for global sample 0 + cell2 for the tracked column;
                stores ct0/z2h/s'track for layer l and advances P track."""
                # cell1 z1 pre-activations
                for cb in range(CB):
                    nc.tensor.matmul(
                        cellps[0:32, Z1TI:Z1TI + 1],
                        wih1t_sb[:, cb * 32:(cb + 1) * 32],
                        P[:, cb * NC9 + B_SH:cb * NC9 + B_SH + 1],
                        start=(cb == 0), stop=(cb == CB - 1))
                if l == 0:
                    nc.vector.memset(cellps[0:32, Z1TH:Z1TH + 1], 0.0)
                else:
                    for cb in range(CB):
                        nc.tensor.matmul(
                            cellps[0:32, Z1TH:Z1TH + 1],
                            whh1t_sb[:, cb * 32:(cb + 1) * 32],
                            ht0[:, cb:cb + 1],
                            start=(cb == 0), stop=(cb == CB - 1))
                nc.scalar.activation(zcat[0:32, 0:1], cellps[0:32, 0:1],
                                     ACTF.Relu, bias=b1_sb[:, 0:1])
                nc.scalar.activation(zcat[32:64, 0:1], cellps[0:32, 1:2],
                                     ACTF.Relu, bias=b1_sb[:, 1:2])
                gates12(zcat[0:65, 0:1], G1, 1)
                nc.scalar.activation(sg1[:, 0:4], cellps[:, G1:G1 + 4],
                                     ACTF.Sigmoid)
                nc.scalar.activation(sg1[:, 4:8], cellps[:, G1 + 4:G1 + 8],
                                     ACTF.Sigmoid)
                nc.scalar.activation(sg1[:, 8:12], cellps[:, G1 + 8:G1 + 12],
                                     ACTF.Tanh)
                ct_new = ct0s[:, l * CB:(l + 1) * CB]
                nc.vector.tensor_tensor(tm1[:, 0:4], sg1[:, 0:4],
                                        sg1[:, 8:12], ALU.mult)
                if l == 0:
                    nc.vector.tensor_copy(ct_new, tm1[:, 0:4])
                else:
                    nc.vector.tensor_tensor(
                        tm1[:, 4:8], sg1[:, 4:8],
                        ct0s[:, (l - 1) * CB:l * CB], ALU.mult)
                    nc.vector.tensor_tensor(ct_new, tm1[:, 0:4],
                                            tm1[:, 4:8], ALU.add)
                nc.scalar.activation(ht0[:, :], ct_new, ACTF.Sigmoid)
                # cell2 hh path from the updated state
                for cb in range(CB):
                    nc.tensor.matmul(
                        cellps[0:32, Z2H:Z2H + 1],
                        whh1t_sb[:, cb * 32:(cb + 1) * 32],
                        ht0[:, cb:cb + 1],
                        start=(cb == 0), stop=(cb == CB - 1))
                nc.scalar.activation(z2hs[:, l:l + 1], cellps[0:32, 2:3],
                                     ACTF.Relu, bias=b1_sb[:, 1:2])
                # cell2 for the tracked column
                nc.vector.tensor_copy(zcat[0:32, 10:11], zcat[0:32, 0:1])
                nc.vector.tensor_copy(zcat[32:64, 10:11], z2hs[:, l:l + 1])
                gates12(zcat[0:65, 10:11], G2, 1)
                cell2_epilogue(
                    l, G2, 1,
                    lambda cb, pooled=False:
                        (P if pooled else s_l[l])[
                            :, cb * NC9 + B_SH:cb * NC9 + B_SH + 1])

            def sample_s_all(b):
                """s for ALL layers of one shard sample, as soon as its
                pooled sums land (pooled evolves closed-form, so no layer
                needs any conv result)."""
                for l in range(num_layers):
                    for cb in range(CB):
                        nc.tensor.matmul(
                            cellps[0:32, Z1B:Z1B + 1],
                            wih1t_sb[:, cb * 32:(cb + 1) * 32],
                            P[:, cb * NC9 + b:cb * NC9 + b + 1],
                            start=(cb == 0), stop=(cb == CB - 1))
                    nc.scalar.activation(zcat[0:32, 1:2],
                                         cellps[0:32, Z1B:Z1B + 1],
                                         ACTF.Relu, bias=b1_sb[:, 0:1])
                    nc.vector.tensor_copy(zcat[32:64, 1:2], z2hs[:, l:l + 1])
                    gates12(zcat[0:65, 1:2], G2, 1)
                    cell2_epilogue(
                        l, G2, 1,
                        lambda cb, pooled=False:
                            (P if pooled else s_l[l])[
                                :, cb * NC9 + b:cb * NC9 + b + 1])

            # ---------------- conv planes ----------------

            evict_ctr = [0]

            def evict_chunk(dst, ps_flat, shaped_dst):
                """copy a finished PSUM chunk out via ACT/DVE/GPSIMD."""
                eng = EVICT_CYCLE[evict_ctr[0] % len(EVICT_CYCLE)]
                evict_ctr[0] += 1
                if eng == "A":
                    nc.scalar.activation(dst, ps_flat if not shaped_dst else
                                         ps_flat.rearrange(
                                             "p (r w) -> p r w", w=W),
                                         ACTF.Copy)
                elif eng == "V":
                    nc.vector.tensor_copy(dst, ps_flat if not shaped_dst else
                                          ps_flat.rearrange(
                                              "p (r w) -> p r w", w=W))
                else:
                    nc.gpsimd.tensor_copy(dst, ps_flat if not shaped_dst else
                                          ps_flat.rearrange(
                                              "p (r w) -> p r w", w=W))

            def pe_plane_taps(l, p):
                b, cb = p // CB, p % CB
                # center tap + combine folded into a per-plane diagonal
                # diag(w_center + s) built on the ACT engine; the whole
                # 9-matmul accumulation group then stays on the PE
                sdiag = sdp.tile([128, 128], F32R, tag="sdiag", name="sdiag")
                nc.scalar.activation(sdiag[:, :], eye_sb[:, :], ACTF.Copy,
                                     scale=scol(l, p))
                chunks = []
                for hf in range(2):
                    r0 = hf * HALF
                    ps = cvp.tile([128, NCHUNK], F32, tag="cps", name="cps")
                    nc.tensor.matmul(ps[:, :], sdiag[:, :],
                                     intr(p, r0, HALF),
                                     start=True, stop=False)
                    for ti, (dy, dx) in enumerate(TAPS8):
                        nc.tensor.matmul(
                            ps[:, :],
                            diag_sb[:, (cb * 8 + ti) * 128:
                                    (cb * 8 + ti + 1) * 128],
                            shifted(p, r0, dy, dx),
                            start=False, stop=(ti == 7))
                    chunks.append(ps)
                return chunks

            def pe_plane_evict(l, p, chunks, last):
                b, cb = p // CB, p % CB
                if last:
                    ost = ostp.tile([128, HW], F32R, tag="ost", name="ost")
                    for hf in range(2):
                        evict_chunk(ost[:, hf * NCHUNK:(hf + 1) * NCHUNK],
                                    chunks[hf][:, :], shaped_dst=False)
                    nc.scalar.dma_start(
                        y_d[b, cb * 128:(cb + 1) * 128, :, :],
                        ost[:, :].rearrange("p (h w) -> p h w", w=W))
                else:
                    for hf in range(2):
                        evict_chunk(intr(p, hf * HALF, HALF),
                                    chunks[hf][:, :], shaped_dst=True)

            xv = flat[:, 0:NP * PLANE].rearrange("p (pl z) -> p pl z",
                                                 z=PLANE)

            def pair_view(p0, dy, dx, dp=CB):
                """[128, 2, 28, 28] window over planes p0 and p0+dp."""
                woff = (1 + dy) * PC + 1 + dx
                return xv[:, p0:p0 + dp + 1:dp,
                          woff:woff + H * PC].rearrange(
                    "p q (r w) -> p q r w", w=PC)[:, :, :, 0:W]

            def v_pair(l, b0, last):
                """two same-cb DVE planes (samples b0, b0+1); STT APs are
                capped at 3 dims so taps stay per-plane."""
                cb = CB - 1
                for k in range(2):
                    p = b0 * CB + k * CB + cb
                    acc = dvp.tile([128, HW], F32R, tag="dacc", name="dacc")
                    av = acc[:, :].rearrange("p (h w) -> p h w", w=W)
                    nc.vector.tensor_scalar(av, intr(p, 0, H), scol(l, p),
                                            None, ALU.mult)
                    for ti, (dy, dx) in enumerate(TAPS8):
                        nc.vector.scalar_tensor_tensor(
                            av, shiftedF(p, dy, dx),
                            w8_sb[:, cb * 8 + ti:cb * 8 + ti + 1], av,
                            ALU.mult, ALU.add)
                    b = p // CB
                    if last:
                        nc.sync.dma_start(
                            y_d[b, cb * 128:(cb + 1) * 128, :, :], av)
                    else:
                        nc.sync.dma_start(intr(p, 0, H), av)

            def vg_plane(l, p, last):
                b, cb = p // CB, p % CB
                if OWNER[p] == "V":
                    raise AssertionError("V planes go through v_pair")
                else:
                    # GPSIMD: no TensorScalarPtr -- weighted taps as two
                    # tensor_tensor passes with the weight broadcast
                    acc = dgp.tile([128, HW], F32R, tag="dacc", name="dacc")
                    av = acc[:, :].rearrange("p (h w) -> p h w", w=W)
                    tmp = dgt.tile([128, HW], F32, tag="gtmp", name="gtmp")
                    tv = tmp[:, :].rearrange("p (h w) -> p h w", w=W)
                    bc = lambda col: col.unsqueeze(-1).broadcast_to(
                        [128, H, W])
                    # keep the whole GPSIMD plane self-contained (center via
                    # tensor_tensor, writeback via SWDGE) so the slow Pool
                    # never back-pressures the ACT or sync-DMA queues
                    nc.gpsimd.tensor_tensor(av, intr(p, 0, H),
                                            bc(scol(l, p)), ALU.mult)
                    for ti, (dy, dx) in enumerate(TAPS8):
                        nc.gpsimd.tensor_tensor(
                            tv, shiftedF(p, dy, dx),
                            bc(w8_sb[:, cb * 8 + ti:cb * 8 + ti + 1]),
                            ALU.mult)
                        nc.gpsimd.tensor_tensor(av, av, tv, ALU.add)
                dma = nc.sync if OWNER[p] == "V" else nc.gpsimd
                if last:
                    dma.dma_start(y_d[b, cb * 128:(cb + 1) * 128, :, :], av)
                elif WRITEBACK_DMA:
                    dma.dma_start(intr(p, 0, H), av)
                else:
                    nc.vector.tensor_copy(intr(p, 0, H), av)

            # ---------------- staging + emission ----------------

            def stage_sample(b):
                stage = sp.tile([128, CB * HW], F32, tag="stage", name="stage")
                nc.sync.dma_start(
                    stage[:, :].rearrange("p (cb hw) -> p cb hw", hw=HW),
                    x_d[b, :, :, :].rearrange("(cb k) h w -> k cb (h w)",
                                              k=128))
                for cb in range(CB):
                    p = b * CB + cb
                    seg = stage[:, cb * HW:(cb + 1) * HW].rearrange(
                        "p (h w) -> p h w", w=W)
                    pcol = P[:, cb * NC9 + b:cb * NC9 + b + 1]
                    if OWNER[p] == "P":
                        nc.scalar.activation(intr(p, 0, H), seg, ACTF.Copy,
                                             accum_out=pcol)
                    else:
                        nc.vector.tensor_scalar(intr(p, 0, H), seg, 1.0, 0.0,
                                                ALU.mult, ALU.add,
                                                accum_out=pcol)

            def emit_body():
                # tracked pooled seed, then the whole track chain (depends
                # only on host-precomputed p0init -> runs during staging)
                with tc.high_priority():
                    nc.vector.tensor_copy(P[:, B_SH::NC9], p0i_sb[:, :])
                    for l in range(num_layers):
                        track_chain(l)

                # Layer 0 overlapped with staging; the whole gate chain
                # for ALL layers is emitted before any bulk DVE/Pool conv
                # work so s never queues behind it (no layer barriers).
                # Keep at most 2 PE planes (4 PSUM chunks) pending
                # eviction: + 2 chunks being diag-seeded stays within the
                # 7-bank conv pool, and the eviction op trails the matmuls
                # far enough that ACT/DVE never block on the PE.
                pend = []

                def flush_pend(n=0, plane=None):
                    while pend and (len(pend) > n or any(
                            q == plane for q, _l, _c in pend)):
                        q, ql, ch = pend.pop(0)
                        pe_plane_evict(ql, q, ch, ql == num_layers - 1)

                def pe_sample_planes(b):
                    # each sample's PE planes flow through all layers,
                    # rotated so a plane's eviction overlaps the sibling
                    # planes' matmuls
                    for l in range(num_layers):
                        for cb in range(CB):
                            p = b * CB + cb
                            if OWNER[p] == "P":
                                flush_pend(2, p)
                                pend.append((p, l, pe_plane_taps(l, p)))

                for b in range(B_SH):
                    with tc.high_priority():
                        stage_sample(b)
                        sample_s_all(b)
                    if b > 0:
                        pe_sample_planes(b - 1)
                pe_sample_planes(B_SH - 1)
                flush_pend(0)

                # DVE/GPSIMD planes layer-major: writeback latency hides
                # across the other planes of the same layer
                for l in range(num_layers):
                    last = l == num_layers - 1
                    for b0 in range(0, B_SH, 2):
                        v_pair(l, b0, last)
                    for b in range(B_SH):
                        for cb in range(CB):
                            p = b * CB + cb
                            if OWNER[p] == "G":
                                vg_plane(l, p, last)

            if iters == 1:
                emit_body()
            else:
                with tc.For_i(0, iters, 1):
                    emit_body()

    nc.compile()
    return nc


def prep_inputs(x, w_ih_l1, b_ih_l1, w_ih_l2, b_ih_l2,
                w_hh_l1, b_hh_l1, w_hh_l2, b_hh_l2, dw_kernel):
    """Host-side prep: per-core input maps (weights replicated)."""
    x = np.ascontiguousarray(np.asarray(x, dtype=np.float32))
    dw = np.asarray(dw_kernel, np.float32).reshape(C, 9)
    taps8_t = [dy * 3 + dx + 4 for (dy, dx) in TAPS8]
    diag = np.zeros((CB, 8, 128, 128), np.float32)
    w8 = np.zeros((128, CB * 8), np.float32)
    idx = np.arange(128)
    for cb in range(CB):
        for ti, t in enumerate(taps8_t):
            diag[cb, ti, idx, idx] = dw[cb * 128:(cb + 1) * 128, t]
            w8[:, cb * 8 + ti] = dw[cb * 128:(cb + 1) * 128, t]
    w4 = dw[:, 4].reshape(CB, 128).T                      # [128, CB]
    ksum = dw.sum(axis=1).reshape(CB, 128).T
    w2cat = np.concatenate(
        [np.asarray(w_ih_l2, np.float32).T,
         np.asarray(w_hh_l2, np.float32).T,
         (np.asarray(b_ih_l2, np.float32)
          + np.asarray(b_hh_l2, np.float32))[None, :]], axis=0)  # [65, 3C]
    common = {
        "diag": diag.reshape(CB * 8 * 128, 128),
        "w8": w8,
        "wih1t": np.ascontiguousarray(
            (np.asarray(w_ih_l1, np.float32) / HW).T),
        "whh1t": np.ascontiguousarray(np.asarray(w_hh_l1, np.float32).T),
        "w2cat": np.ascontiguousarray(w2cat),
        "b1": np.ascontiguousarray(np.stack(
            [np.asarray(b_ih_l1, np.float32),
             np.asarray(b_hh_l1, np.float32)], axis=1)),
        "ksw": np.ascontiguousarray(ksum - w4),           # s + ksum = s' + ksw
        "w4p1": np.ascontiguousarray(1.0 + w4),           # s' = sig + w4p1
        "p0init": np.ascontiguousarray(
            x[0].reshape(C, HW).sum(axis=1).reshape(CB, 128).T),
        "eye": np.eye(128, dtype=np.float32),
    }
    return [dict(common, x=np.ascontiguousarray(x[i * B_SH:(i + 1) * B_SH]))
            for i in range(N_CORES)]


_cache = {}


def kernel(**inputs) -> np.ndarray:
    num_layers = int(inputs["num_layers"])
    if num_layers == 0:
        return np.asarray(inputs["x"], np.float32).copy()
    if num_layers not in _cache:
        _cache[num_layers] = build_program(num_layers=num_layers, iters=1)
    nc = _cache[num_layers]
    in_maps = prep_inputs(
        inputs["x"], inputs["w_ih_l1"], inputs["b_ih_l1"], inputs["w_ih_l2"],
        inputs["b_ih_l2"], inputs["w_hh_l1"], inputs["b_hh_l1"],
        inputs["w_hh_l2"], inputs["b_hh_l2"], inputs["dw_kernel"])
    res = run_bass_kernel_spmd(nc, in_maps, list(range(N_CORES)))
    return np.concatenate([res.results[i]["y"] for i in range(N_CORES)],
                          axis=0).astype(np.float32)
